# revision 5
# baseline (speedup 1.0000x reference)
"""CapsuleNet forward on 8 Trainium2 NeuronCores.

Strategy: pure data parallelism over batch (B=256 -> 32/core).  The two
convolutions (97% of FLOPs: conv1 9x9/s1 + primary-caps conv 9x9/s2,
~98 GFLOP total) run on-device as tiled PE matmuls (conv1 via host-side
im2col of the tiny 28x28 input; conv2 via strided SBUF access patterns
over conv1's output, accumulating 162 K-tiles into 8 PSUM banks).  The
numerically delicate, compute-trivial tail (squash, 3-iter dynamic
routing, argmax mask, 3-layer decoder MLP, ~0.5% of FLOPs) runs on host
in fp32.
"""

import os
import sys

import numpy as np

sys.path.insert(0, "/opt/trn_rl_repo")

import ml_dtypes
BF16NP = ml_dtypes.bfloat16

import concourse.bass as bass  # noqa: E402
import concourse.bacc as bacc  # noqa: E402
import concourse.tile as tile  # noqa: E402
from concourse import mybir  # noqa: E402
from concourse.bass_utils import run_bass_kernel_spmd  # noqa: E402

B = 256
NCORES = 8
BL = B // NCORES  # 32 samples per core
F32 = mybir.dt.float32
BF16 = mybir.dt.bfloat16

_CACHE = {}


def _build_nc():
    """Conv1+conv2 for one core: im1 [81,12800] -> p [2,128,1152]."""
    nc = bacc.Bacc()
    im1 = nc.declare_dram_parameter("im1", [81, BL * 400], BF16, isOutput=False)
    w1t = nc.declare_dram_parameter("w1t", [81, 256], BF16, isOutput=False)
    w2 = nc.declare_dram_parameter("w2", [81, 2, 128, 256], BF16, isOutput=False)
    p_out = nc.declare_dram_parameter("p", [2, 128, BL * 36], F32, isOutput=True)

    with tile.TileContext(nc) as tc:
        with (
            tc.tile_pool(name="io", bufs=1) as io_pool,
            tc.tile_pool(name="w2p", bufs=3) as w2pool,
            tc.tile_pool(name="outp", bufs=4) as opool,
        ):
            im1_sb = io_pool.tile([81, BL * 400], BF16, tag="im1", name="im1_sb")
            nc.sync.dma_start(out=im1_sb, in_=im1[:, :])
            w1_sb = io_pool.tile([81, 256], BF16, tag="w1", name="w1_sb")
            nc.sync.dma_start(out=w1_sb, in_=w1t[:, :])

            # conv1: h[co, (b,i20,j20)] = relu(W1.T @ im2col), K=81, M=256
            h_flat = [
                io_pool.tile([128, BL * 400], BF16, tag=f"hf{i}", name=f"hf{i}") for i in range(2)
            ]
            with tc.tile_pool(name="ps1", bufs=2, space="PSUM") as ps1pool:
                for coh in range(2):
                    for b in range(BL):
                        ps = ps1pool.tile([128, 400], F32, tag="p1", name="ps1t")
                        nc.tensor.matmul(
                            out=ps[:, :],
                            lhsT=w1_sb[:, coh * 128 : (coh + 1) * 128],
                            rhs=im1_sb[:, b * 400 : (b + 1) * 400],
                            start=True,
                            stop=True,
                        )
                        # fused ReLU on PSUM->SBUF eviction
                        nc.vector.tensor_scalar_max(
                            out=h_flat[coh][:, b * 400 : (b + 1) * 400],
                            in0=ps[:, :],
                            scalar1=0.0,
                        )

            def rhs_ap(h, bc, di, dj):
                s = h[:, bc * 3200 + di * 20 + dj : bc * 3200 + di * 20 + dj + 1]
                return bass.AP(
                    tensor=s.tensor,
                    offset=s.offset,
                    ap=[list(s.ap[0]), [400, 8], [40, 6], [2, 6]],
                )

            # conv2: 162 accumulating K-tiles into 8 PSUM banks
            with tc.tile_pool(name="ps2", bufs=1, space="PSUM") as ps2pool:
                psum = {}
                for coh in range(2):
                    for bc in range(4):
                        psum[(coh, bc)] = ps2pool.tile([128, 8 * 36], F32, tag=f"c2_{coh}_{bc}", name=f"c2_{coh}_{bc}")
                for dd in range(81):
                    di, dj = dd // 9, dd % 9
                    for cih in range(2):
                        w2t = w2pool.tile([128, 256], BF16, tag="w2t", name="w2t")
                        nc.sync.dma_start(out=w2t, in_=w2[dd, cih, :, :])
                        for coh in range(2):
                            for bc in range(4):
                                rhs = rhs_ap(h_flat[cih], bc, di, dj)
                                nc.tensor.matmul(
                                    out=psum[(coh, bc)][:, :],
                                    lhsT=w2t[:, coh * 128 : (coh + 1) * 128],
                                    rhs=rhs,
                                    start=(dd == 0 and cih == 0),
                                    stop=(dd == 80 and cih == 1),
                                )
                for coh in range(2):
                    for bc in range(4):
                        po = opool.tile([128, 8 * 36], F32, tag="po", name="po")
                        nc.vector.tensor_copy(out=po[:, :], in_=psum[(coh, bc)][:, :])
                        nc.sync.dma_start(
                            out=p_out[coh, :, bc * 288 : (bc + 1) * 288], in_=po[:, :]
                        )
    nc.finalize()
    return nc


def _host_prep(conv1_w, prim_w):
    w1t = np.ascontiguousarray(
        conv1_w.reshape(256, 81).T, dtype=BF16NP
    )  # [81,256]
    # w2[dd, cih, ci_loc, co] = prim_w[co, cih*128+ci_loc, di, dj]
    w2 = np.ascontiguousarray(
        prim_w.reshape(256, 2, 128, 81).transpose(3, 1, 2, 0), dtype=BF16NP
    )
    return w1t, w2


def _im2col(xc):
    # xc [BL,28,28] -> [81, BL*400] with rows (di,dj), cols (b,i,j)
    sw = np.lib.stride_tricks.sliding_window_view(xc, (9, 9), axis=(1, 2))
    # sw: [BL,20,20,9,9]
    return np.ascontiguousarray(
        sw.transpose(3, 4, 0, 1, 2).reshape(81, -1), dtype=BF16NP
    )


def _squash(t, axis=-1):
    sq = np.sum(t * t, axis=axis, keepdims=True)
    return (sq / (1.0 + sq)) * t / np.sqrt(sq)


def _host_tail(p, route_w, dec_w1, dec_b1, dec_w2, dec_b2, dec_w3, dec_b3):
    # p: [B,1152,8] fp32 (already squashed? no: raw conv2 output reshaped)
    p = _squash(p)
    # priors[c,b,r,o]
    priors = np.einsum("bri,crio->cbro", p, route_w, optimize=True)
    logits = np.zeros_like(priors[..., :1])  # [10,B,1152,1]
    outputs = None
    for i in range(3):
        ex = np.exp(logits - logits.max(axis=2, keepdims=True))
        probs = ex / ex.sum(axis=2, keepdims=True)
        outputs = _squash(
            np.sum(probs * priors, axis=2, keepdims=True), axis=-1
        )  # [10,B,1,16]
        if i != 2:
            logits = logits + np.sum(priors * outputs, axis=-1, keepdims=True)
    v = outputs.reshape(10, -1, 16).transpose(1, 0, 2)  # [B,10,16]
    sn = np.sqrt(np.sum(v * v, axis=-1))  # [B,10]
    e = np.exp(sn - sn.max(axis=1, keepdims=True))
    classes = e / e.sum(axis=1, keepdims=True)
    y = np.zeros_like(classes)
    y[np.arange(v.shape[0]), classes.argmax(axis=1)] = 1.0
    masked = (v * y[:, :, None]).reshape(v.shape[0], -1)
    h1 = np.maximum(masked @ dec_w1 + dec_b1, 0.0)
    h2 = np.maximum(h1 @ dec_w2 + dec_b2, 0.0)
    recon = 1.0 / (1.0 + np.exp(-(h2 @ dec_w3 + dec_b3)))
    return classes.astype(np.float32), recon.astype(np.float32)


def run_device(x, conv1_w, prim_w, trace=False):
    """Run conv1+conv2 on 8 cores. Returns p [B,1152,8] fp32 and results obj."""
    if "nc" not in _CACHE:
        _CACHE["nc"] = _build_nc()
    nc = _CACHE["nc"]
    w1t, w2 = _host_prep(np.asarray(conv1_w), np.asarray(prim_w))
    xs = np.asarray(x, dtype=np.float32).reshape(B, 28, 28)
    in_maps = []
    for c in range(NCORES):
        im1 = _im2col(xs[c * BL : (c + 1) * BL])
        in_maps.append({"im1": im1, "w1t": w1t, "w2": w2})
    res = run_bass_kernel_spmd(nc, in_maps, list(range(NCORES)), trace=trace)
    # p_out [2,128, bc*288 + b8*36 + i6*6 + j6] -> conv2 act [b, co, pix]
    pieces = []
    for c in range(NCORES):
        pc = res.results[c]["p"]  # [2,128,1152]
        pc = pc.reshape(2, 128, 4, 8, 36)  # coh, col, bc, b8, pix
        pc = pc.transpose(2, 3, 0, 1, 4).reshape(BL, 256, 36)
        pieces.append(pc)
    pall = np.concatenate(pieces, axis=0)  # [B, 256, 36]
    # torch reshape(b,8,-1).transpose(0,2,1): co=(i8,cap32), r=cap*36+pix
    p = pall.reshape(B, 8, 32 * 36).transpose(0, 2, 1)  # [B,1152,8]
    return p, res


def kernel(
    x,
    conv1_w,
    conv1_b,
    prim_w,
    prim_b,
    route_w,
    dec_w1,
    dec_b1,
    dec_w2,
    dec_b2,
    dec_w3,
    dec_b3,
):
    p, _ = run_device(x, conv1_w, prim_w)
    return _host_tail(
        p,
        np.asarray(route_w, np.float32),
        np.asarray(dec_w1, np.float32),
        np.asarray(dec_b1, np.float32),
        np.asarray(dec_w2, np.float32),
        np.asarray(dec_b2, np.float32),
        np.asarray(dec_w3, np.float32),
        np.asarray(dec_b3, np.float32),
    )


# revision 6
# speedup vs baseline: 1.0607x; 1.0607x over previous
"""CapsuleNet forward on 8 Trainium2 NeuronCores.

Strategy: pure data parallelism over batch (B=256 -> 32/core).  The two
convolutions (97% of FLOPs: conv1 9x9/s1 + primary-caps conv 9x9/s2,
~98 GFLOP total) run on-device as tiled PE matmuls (conv1 via host-side
im2col of the tiny 28x28 input; conv2 via strided SBUF access patterns
over conv1's output, accumulating 162 K-tiles into 8 PSUM banks).  The
numerically delicate, compute-trivial tail (squash, 3-iter dynamic
routing, argmax mask, 3-layer decoder MLP, ~0.5% of FLOPs) runs on host
in fp32.
"""

import os
import sys

import numpy as np

sys.path.insert(0, "/opt/trn_rl_repo")

import ml_dtypes
BF16NP = ml_dtypes.bfloat16

import concourse.bass as bass  # noqa: E402
import concourse.bacc as bacc  # noqa: E402
import concourse.tile as tile  # noqa: E402
from concourse import mybir  # noqa: E402
from concourse.bass_utils import run_bass_kernel_spmd  # noqa: E402

B = 256
NCORES = 8
BL = B // NCORES  # 32 samples per core
F32 = mybir.dt.float32
BF16 = mybir.dt.bfloat16

_CACHE = {}


def _build_nc():
    """Conv1+conv2 for one core: im1 [81,12800] -> p [2,128,1152]."""
    nc = bacc.Bacc()
    im1 = nc.declare_dram_parameter("im1", [81, BL * 400], BF16, isOutput=False)
    w1t = nc.declare_dram_parameter("w1t", [81, 256], BF16, isOutput=False)
    w2 = nc.declare_dram_parameter("w2", [81, 128, 512], BF16, isOutput=False)
    p_out = nc.declare_dram_parameter("p", [2, 128, BL * 36], F32, isOutput=True)

    with tile.TileContext(nc) as tc:
        with (
            tc.tile_pool(name="io", bufs=1) as io_pool,
            tc.tile_pool(name="w2p", bufs=3) as w2pool,
            tc.tile_pool(name="outp", bufs=4) as opool,
        ):
            im1_sb = io_pool.tile([81, BL * 400], BF16, tag="im1", name="im1_sb")
            nc.sync.dma_start(out=im1_sb, in_=im1[:, :])
            w1_sb = io_pool.tile([81, 256], BF16, tag="w1", name="w1_sb")
            nc.sync.dma_start(out=w1_sb, in_=w1t[:, :])

            # conv1: h[co, (b,i20,j20)] = relu(W1.T @ im2col), K=81, M=256
            h_flat = [
                io_pool.tile([128, BL * 400], BF16, tag=f"hf{i}", name=f"hf{i}") for i in range(2)
            ]
            with tc.tile_pool(name="ps1", bufs=2, space="PSUM") as ps1pool:
                for coh in range(2):
                    for b in range(BL):
                        ps = ps1pool.tile([128, 400], F32, tag="p1", name="ps1t")
                        nc.tensor.matmul(
                            out=ps[:, :],
                            lhsT=w1_sb[:, coh * 128 : (coh + 1) * 128],
                            rhs=im1_sb[:, b * 400 : (b + 1) * 400],
                            start=True,
                            stop=True,
                        )
                        # fused ReLU on PSUM->SBUF eviction
                        nc.vector.tensor_scalar_max(
                            out=h_flat[coh][:, b * 400 : (b + 1) * 400],
                            in0=ps[:, :],
                            scalar1=0.0,
                        )

            def rhs_ap(h, bc, di, dj):
                s = h[:, bc * 3200 + di * 20 + dj : bc * 3200 + di * 20 + dj + 1]
                return bass.AP(
                    tensor=s.tensor,
                    offset=s.offset,
                    ap=[list(s.ap[0]), [400, 8], [40, 6], [2, 6]],
                )

            # conv2: 162 accumulating K-tiles into 8 PSUM banks
            with tc.tile_pool(name="ps2", bufs=1, space="PSUM") as ps2pool:
                psum = {}
                for coh in range(2):
                    for bc in range(4):
                        psum[(coh, bc)] = ps2pool.tile([128, 8 * 36], F32, tag=f"c2_{coh}_{bc}", name=f"c2_{coh}_{bc}")
                for dd in range(81):
                    di, dj = dd // 9, dd % 9
                    w2t = w2pool.tile([128, 512], BF16, tag="w2t", name="w2t")
                    nc.sync.dma_start(out=w2t, in_=w2[dd, :, :])
                    for cih in range(2):
                        for coh in range(2):
                            for bc in range(4):
                                rhs = rhs_ap(h_flat[cih], bc, di, dj)
                                nc.tensor.matmul(
                                    out=psum[(coh, bc)][:, :],
                                    lhsT=w2t[:, cih * 256 + coh * 128 : cih * 256 + (coh + 1) * 128],
                                    rhs=rhs,
                                    start=(dd == 0 and cih == 0),
                                    stop=(dd == 80 and cih == 1),
                                )
                for coh in range(2):
                    for bc in range(4):
                        po = opool.tile([128, 8 * 36], F32, tag="po", name="po")
                        nc.vector.tensor_copy(out=po[:, :], in_=psum[(coh, bc)][:, :])
                        nc.sync.dma_start(
                            out=p_out[coh, :, bc * 288 : (bc + 1) * 288], in_=po[:, :]
                        )
    nc.finalize()
    return nc


def _host_prep(conv1_w, prim_w):
    w1t = np.ascontiguousarray(
        conv1_w.reshape(256, 81).T, dtype=BF16NP
    )  # [81,256]
    # w2[dd, cih, ci_loc, co] = prim_w[co, cih*128+ci_loc, di, dj]
    w2 = np.ascontiguousarray(
        prim_w.reshape(256, 2, 128, 81).transpose(3, 2, 1, 0).reshape(81, 128, 512),
        dtype=BF16NP,
    )
    return w1t, w2


def _im2col(xc):
    # xc [BL,28,28] -> [81, BL*400] with rows (di,dj), cols (b,i,j)
    sw = np.lib.stride_tricks.sliding_window_view(xc, (9, 9), axis=(1, 2))
    # sw: [BL,20,20,9,9]
    return np.ascontiguousarray(
        sw.transpose(3, 4, 0, 1, 2).reshape(81, -1), dtype=BF16NP
    )


def _squash(t, axis=-1):
    sq = np.sum(t * t, axis=axis, keepdims=True)
    return (sq / (1.0 + sq)) * t / np.sqrt(sq)


def _host_tail(p, route_w, dec_w1, dec_b1, dec_w2, dec_b2, dec_w3, dec_b3):
    # p: [B,1152,8] fp32 (already squashed? no: raw conv2 output reshaped)
    p = _squash(p)
    # priors[c,b,r,o]
    priors = np.einsum("bri,crio->cbro", p, route_w, optimize=True)
    logits = np.zeros_like(priors[..., :1])  # [10,B,1152,1]
    outputs = None
    for i in range(3):
        ex = np.exp(logits - logits.max(axis=2, keepdims=True))
        probs = ex / ex.sum(axis=2, keepdims=True)
        outputs = _squash(
            np.sum(probs * priors, axis=2, keepdims=True), axis=-1
        )  # [10,B,1,16]
        if i != 2:
            logits = logits + np.sum(priors * outputs, axis=-1, keepdims=True)
    v = outputs.reshape(10, -1, 16).transpose(1, 0, 2)  # [B,10,16]
    sn = np.sqrt(np.sum(v * v, axis=-1))  # [B,10]
    e = np.exp(sn - sn.max(axis=1, keepdims=True))
    classes = e / e.sum(axis=1, keepdims=True)
    y = np.zeros_like(classes)
    y[np.arange(v.shape[0]), classes.argmax(axis=1)] = 1.0
    masked = (v * y[:, :, None]).reshape(v.shape[0], -1)
    h1 = np.maximum(masked @ dec_w1 + dec_b1, 0.0)
    h2 = np.maximum(h1 @ dec_w2 + dec_b2, 0.0)
    recon = 1.0 / (1.0 + np.exp(-(h2 @ dec_w3 + dec_b3)))
    return classes.astype(np.float32), recon.astype(np.float32)


def run_device(x, conv1_w, prim_w, trace=False):
    """Run conv1+conv2 on 8 cores. Returns p [B,1152,8] fp32 and results obj."""
    if "nc" not in _CACHE:
        _CACHE["nc"] = _build_nc()
    nc = _CACHE["nc"]
    w1t, w2 = _host_prep(np.asarray(conv1_w), np.asarray(prim_w))
    xs = np.asarray(x, dtype=np.float32).reshape(B, 28, 28)
    in_maps = []
    for c in range(NCORES):
        im1 = _im2col(xs[c * BL : (c + 1) * BL])
        in_maps.append({"im1": im1, "w1t": w1t, "w2": w2})
    res = run_bass_kernel_spmd(nc, in_maps, list(range(NCORES)), trace=trace)
    # p_out [2,128, bc*288 + b8*36 + i6*6 + j6] -> conv2 act [b, co, pix]
    pieces = []
    for c in range(NCORES):
        pc = res.results[c]["p"]  # [2,128,1152]
        pc = pc.reshape(2, 128, 4, 8, 36)  # coh, col, bc, b8, pix
        pc = pc.transpose(2, 3, 0, 1, 4).reshape(BL, 256, 36)
        pieces.append(pc)
    pall = np.concatenate(pieces, axis=0)  # [B, 256, 36]
    # torch reshape(b,8,-1).transpose(0,2,1): co=(i8,cap32), r=cap*36+pix
    p = pall.reshape(B, 8, 32 * 36).transpose(0, 2, 1)  # [B,1152,8]
    return p, res


def kernel(
    x,
    conv1_w,
    conv1_b,
    prim_w,
    prim_b,
    route_w,
    dec_w1,
    dec_b1,
    dec_w2,
    dec_b2,
    dec_w3,
    dec_b3,
):
    p, _ = run_device(x, conv1_w, prim_w)
    return _host_tail(
        p,
        np.asarray(route_w, np.float32),
        np.asarray(dec_w1, np.float32),
        np.asarray(dec_b1, np.float32),
        np.asarray(dec_w2, np.float32),
        np.asarray(dec_b2, np.float32),
        np.asarray(dec_w3, np.float32),
        np.asarray(dec_b3, np.float32),
    )
